# revision 11
# baseline (speedup 1.0000x reference)
"""Trainium2 Bass kernel for nn_Attention_2 (B=32, LQ=LK=2048, H=1024, A=512).

Math (q-sum distributes through the matmul, so [B,LQ,LK] never exists):
  qs[b]   = sum_q query[b,q,:]
  qp[b]   = qs[b] @ Wq + LQ*bq
  u[b]    = qp[b] @ Wk^T            (score weights, [H])
  w       = Wk @ Wv[:,0]            (v weights, [H])
  s[b,k]  = key[b,k,:] . u[b]  (+ qp.bk const, cancels in softmax)
  v[b,k]  = key[b,k,:] . w     (+ cv = bk.Wv + bv, folded at end)
  x[b]    = softmax(s) . v + cv

v7 architecture (vs the DMA-accumulate baseline, ~367us):
  - All DMA is plain packets on the two hardware queues.  One hw queue
    sustains ~378 GB/s (measured); accumulate packets run the SDMA engines
    at half rate, so the query reduction moved on-chip (PE ones-matmuls).
  - Query streams batch-sequentially (order b3,b0,b1,b2) on the ACT queue;
    each batch's prep (qs->qsT->qp->qpT->u->w2/wqbc) runs as soon as its
    last tile lands, so that batch's key dots start while later batches'
    query still streams.
  - Key streams in batch-interleaved granule waves (256 rows / 1MB):
    b3/b0/b1 on the SYNC queue; b2 on the ACT queue split around prep2 so
    a ring-full dma_start can never park the ACT stream ahead of the
    copies that would free it (deadlock audit).
  - b3 takes the PE route: fp32 transposes -> ACT copy (rounds to f32r)
    -> f32r matmuls with the [uT|wT] stationary pair (f32r = 1 cyc/col
    for moving dims >=256 vs 4 for fp32; 11 mantissa bits -> measured
    end-to-end rel-err ~2.6e-3, gate 2e-2).  b0..b2 take the DVE route
    (fused mul+reduce STT rowwise dots, fp32).
  - Raw Wk/Wq stage through the DVE key ring (consumed in the first ~15us)
    so only the rounded/transposed copies hold permanent SBUF.
"""
import numpy as np

import concourse.bass as bass
import concourse.bacc as bacc
import concourse.tile as tile
from concourse import mybir
from concourse.bass_utils import run_bass_kernel_spmd

N_CORES = 8
B, LQ, LK, H, A = 32, 2048, 2048, 1024, 512
BPC = B // N_CORES
P = 128
f32 = mybir.dt.float32
f32r = mybir.dt.float32r
NG = 8                      # key granules per batch (256 rows each)
GR = LK // NG               # 256
NQD = 8                     # query DMAs per batch ([128, 2H] tiles)
HJ = H // P                 # 8
AC = A // P                 # 4

_CACHE = {}
import os as _os
QLAND_BUFS = int(_os.environ.get("QLAND_BUFS", "3"))
KTPE_BUFS = int(_os.environ.get("KTPE_BUFS", "3"))
KTD01_BUFS = int(_os.environ.get("KTD01_BUFS", "4"))
KTD2_BUFS = int(_os.environ.get("KTD2_BUFS", "3"))
KEYT_BUFS = int(_os.environ.get("KEYT_BUFS", "2"))
QFOLD = int(_os.environ.get("QFOLD", "1"))
PREP_DMA = int(_os.environ.get("PREP_DMA", "1"))


def build_bass():
    nc = bacc.Bacc(None, target_bir_lowering=False, debug=False)

    query = nc.dram_tensor("query", [BPC, LQ, H], f32, kind="ExternalInput").ap()
    key = nc.dram_tensor("key", [BPC, LK, H], f32, kind="ExternalInput").ap()
    Wq = nc.dram_tensor("Wq", [H, A], f32, kind="ExternalInput").ap()
    bq = nc.dram_tensor("bq", [A], f32, kind="ExternalInput").ap()
    Wk = nc.dram_tensor("Wk", [H, A], f32, kind="ExternalInput").ap()
    bk = nc.dram_tensor("bk", [A], f32, kind="ExternalInput").ap()
    Wv = nc.dram_tensor("Wv", [A, 1], f32, kind="ExternalInput").ap()
    bv = nc.dram_tensor("bv", [1], f32, kind="ExternalInput").ap()
    out = nc.dram_tensor("out", [BPC, 1], f32, kind="ExternalOutput").ap()
    scr_qs = nc.dram_tensor("scr_qs", [H], f32, kind="Internal").ap()
    scr_qp = nc.dram_tensor("scr_qp", [A], f32, kind="Internal").ap()
    scr_u = nc.dram_tensor("scr_u", [H], f32, kind="Internal").ap()

    with tile.TileContext(nc) as tc:
        _build_body(nc, tc, query, key, Wq, bq, Wk, bk, Wv, bv, out,
                    scr_qs, scr_qp, scr_u)
    nc.compile()
    return nc


def _build_body(nc, tc, query, key, Wq, bq, Wk, bk, Wv, bv, out,
                scr_qs, scr_qp, scr_u):
    from contextlib import ExitStack
    ctx = ExitStack()
    with ctx:
        sbc = ctx.enter_context(tc.tile_pool(name="sbc", bufs=1))
        sbq = ctx.enter_context(tc.tile_pool(name="sbq", bufs=1))
        sbk = ctx.enter_context(tc.tile_pool(name="sbk", bufs=1))
        sbr = ctx.enter_context(tc.tile_pool(name="sbr", bufs=1))
        sbsv = ctx.enter_context(tc.tile_pool(name="sbsv", bufs=1))
        sbj = ctx.enter_context(tc.tile_pool(name="sbj", bufs=1))
        sbsm = ctx.enter_context(tc.tile_pool(name="sbsm", bufs=1))
        ps_qr = ctx.enter_context(tc.tile_pool(name="ps_qr", bufs=1, space="PSUM"))
        ps_kt = ctx.enter_context(tc.tile_pool(name="ps_kt", bufs=2, space="PSUM"))
        ps_s2 = ctx.enter_context(tc.tile_pool(name="ps_s2", bufs=2, space="PSUM"))
        ps_sm = ctx.enter_context(tc.tile_pool(name="ps_sm", bufs=1, space="PSUM"))

        # ---------------- constants ----------------
        ident = sbc.tile([P, P], f32)
        colidx = sbsm.tile([P, P], f32, tag="small")
        rowidx = sbsm.tile([P, 1], f32, tag="tiny")
        nc.gpsimd.iota(colidx[:], pattern=[[1, P]], base=0, channel_multiplier=0,
                       allow_small_or_imprecise_dtypes=True)
        nc.gpsimd.iota(rowidx[:], pattern=[[0, 1]], base=0, channel_multiplier=1,
                       allow_small_or_imprecise_dtypes=True)
        nc.vector.tensor_scalar(out=ident[:], in0=colidx[:], scalar1=rowidx[:],
                                scalar2=None, op0=mybir.AluOpType.is_equal)
        ones1 = sbc.tile([P, 1], f32)
        nc.vector.memset(ones1[:], 1.0)
        one11 = sbc.tile([1, 1], f32)
        nc.vector.memset(one11[:], 1.0)
        ones_k1 = sbc.tile([1, P], f32)
        nc.vector.memset(ones_k1[:], 1.0)

        # -------- weight DMAs (ACT queue, first) --------
        # raw Wk/Wq stage through the ktd01 ring (slots recycled for keys)
        wk_st = [sbk.tile([P, 4 * A], f32, tag="ktd01", bufs=KTD01_BUFS,
                          name=f"wkst{i}") for i in range(2)]
        for i in range(2):
            nc.scalar.dma_start(
                out=wk_st[i][:].rearrange("p (j a) -> p j a", j=4),
                in_=Wk[i * A:(i + 1) * A, :].rearrange("(j p) a -> p j a", p=P))
        wq_st = [sbk.tile([P, 4 * A], f32, tag="ktd01", bufs=KTD01_BUFS,
                          name=f"wqst{i}") for i in range(2)]
        for i in range(2):
            nc.scalar.dma_start(
                out=wq_st[i][:].rearrange("p (j a) -> p j a", j=4),
                in_=Wq[i * A:(i + 1) * A, :].rearrange("(j p) a -> p j a", p=P))
        wv_sb = sbc.tile([P, AC], f32)
        nc.scalar.dma_start(out=wv_sb[:].rearrange("p (c o) -> p c o", c=AC),
                            in_=Wv.rearrange("(c p) o -> p c o", p=P))
        bk_sb = sbc.tile([P, AC], f32)
        nc.scalar.dma_start(out=bk_sb[:], in_=bk.rearrange("(c p) -> p c", p=P))
        bv_sb = sbc.tile([1, 1], f32)
        nc.scalar.dma_start(out=bv_sb[:], in_=bv[None, :])
        bq_row = sbc.tile([1, A], f32)
        nc.scalar.dma_start(out=bq_row[:], in_=bq[None, :])

        # -------- one-time weight prep (PE+ACT+DVE, before query issues) ----
        # WkT_r [a-part, (c h)] f32r, rounded at the PSUM->SBUF copies
        WkT_r = sbc.tile([P, AC * H], f32r)
        for c in range(AC):
            for half in range(2):
                wkp = ps_kt.tile([P, A], f32, tag="ktp")
                for jl in range(4):
                    nc.tensor.transpose(
                        wkp[:, jl * P:(jl + 1) * P],
                        wk_st[half][:, jl * A + c * P:jl * A + (c + 1) * P],
                        ident[:])
                nc.scalar.copy(
                    WkT_r[:, c * H + half * A:c * H + (half + 1) * A], wkp[:])
        # Wq_r [h-part, (j a)] f32r
        Wq_r = sbc.tile([P, HJ * A], f32r)
        for i in range(2):
            nc.scalar.copy(Wq_r[:, i * 4 * A:(i + 1) * 4 * A], wq_st[i][:])
        # wv_r, then w row = Wv^T @ WkT  [1, H] fp32
        wv_r = sbc.tile([P, AC], f32r)
        nc.scalar.copy(wv_r[:], wv_sb[:])
        w_sb = sbr.tile([1, H], f32, tag="u", bufs=1, name="w_sb")
        for half in range(2):
            w_ps = ps_sm.tile([1, A], f32, tag="small")
            for c in range(AC):
                nc.tensor.matmul(w_ps[:], wv_r[:, c:c + 1],
                                 WkT_r[:, c * H + half * A:c * H + (half + 1) * A],
                                 start=(c == 0), stop=(c == AC - 1))
            nc.scalar.copy(w_sb[:, half * A:(half + 1) * A], w_ps[:])
        # wvbc [P, H] = broadcast w across partitions (DVE route operand)
        wvbc = sbc.tile([P, H], f32)
        for half in range(2):
            bc_ps = ps_sm.tile([P, A], f32, tag="small")
            nc.tensor.matmul(bc_ps[:], ones_k1[:], w_sb[:, half * A:(half + 1) * A],
                             start=True, stop=True)
            nc.scalar.copy(wvbc[:, half * A:(half + 1) * A], bc_ps[:])
        # wT8 [128, 8]: w chunks as columns (for the b3 w2 tile)
        wT8_ps = ps_sm.tile([P, HJ], f32, tag="small")
        for j in range(HJ):
            nc.tensor.matmul(wT8_ps[:, j:j + 1], w_sb[:, j * P:(j + 1) * P],
                             one11[:], start=True, stop=True)
        wT8 = sbc.tile([P, HJ], f32)
        nc.scalar.copy(wT8[:], wT8_ps[:])
        # cv = bk . Wv + bv
        junk4 = sbsm.tile([P, AC], f32, tag="tiny2")
        cvcol = sbsm.tile([P, 1], f32, tag="tiny3")
        nc.vector.scalar_tensor_tensor(out=junk4[:], in0=bk_sb[:], scalar=1.0,
                                       in1=wv_sb[:], op0=mybir.AluOpType.mult,
                                       op1=mybir.AluOpType.mult, accum_out=cvcol[:])
        cv_ps = ps_sm.tile([1, 1], f32, tag="small")
        nc.tensor.matmul(cv_ps[:], cvcol[:], ones1[:], start=True, stop=True)
        cv_sb = sbc.tile([1, 1], f32)
        nc.vector.tensor_tensor(out=cv_sb[:], in0=cv_ps[:], in1=bv_sb[:],
                                op=mybir.AluOpType.add)

        # -------- key DMAs for b3/b0/b1 (SYNC queue, granule waves) --------
        kt = {}
        for g in range(NG):
            for b in (3, 0, 1):
                tag, bufs = ("ktpe", KTPE_BUFS) if b == 3 else ("ktd01", KTD01_BUFS)
                t = sbk.tile([P, 2 * H], f32, tag=tag, bufs=bufs, name=f"k{b}_{g}")
                nc.sync.dma_start(
                    out=t[:].rearrange("p (c h) -> p c h", c=2),
                    in_=key[b, g * GR:(g + 1) * GR, :]
                    .rearrange("(c p) h -> p c h", p=P))
                kt[(b, g)] = t

        # ---------------- per-batch state ----------------
        preps = {b: {} for b in range(BPC)}
        sv3 = sbsv.tile([2, LK], f32, tag="sv3")
        sdve = {b: sbsv.tile([P, 2 * NG], f32, tag=f"sd{b}", name=f"sdve{b}") for b in range(3)}
        vdve = {b: sbsv.tile([P, 2 * NG], f32, tag=f"vd{b}", name=f"vdve{b}") for b in range(3)}
        qland = {}

        def emit_qdma(b, lo, hi):
            if lo == 0:
                preps[b]["qr_ps"] = [ps_qr.tile([1, A], f32, tag=f"qr{h}", bufs=1,
                                                 name=f"qrps{b}_{h}")
                                     for h in range(2)]
            for i in range(lo, hi):
                t = sbq.tile([P, 2 * H], f32, tag="qland", bufs=QLAND_BUFS,
                             name=f"q{b}_{i}")
                nc.scalar.dma_start(
                    out=t[:].rearrange("p (c h) -> p c h", c=2),
                    in_=query[b, i * 2 * P:(i + 1) * 2 * P, :]
                    .rearrange("(c p) h -> p c h", p=P))
                qland[(b, i)] = t

        def emit_kdma2(glo, ghi):
            """b2 key granules on the ACT queue (split around prep2)."""
            for g in range(glo, ghi):
                t = sbk.tile([P, 2 * H], f32, tag="ktd2", bufs=KTD2_BUFS,
                             name=f"k2_{g}")
                nc.scalar.dma_start(
                    out=t[:].rearrange("p (c h) -> p c h", c=2),
                    in_=key[2, g * GR:(g + 1) * GR, :]
                    .rearrange("(c p) h -> p c h", p=P))
                kt[(2, g)] = t

        def emit_qfold(b, lo, hi):
            """GpSimd pair-folds: tile 2i+1 += tile 2i (in place)."""
            for i in range(lo, hi, 2):
                t0, t1 = qland[(b, i)], qland[(b, i + 1)]
                nc.gpsimd.tensor_tensor(out=t1[:], in0=t0[:], in1=t1[:],
                                        op=mybir.AluOpType.add)

        def emit_qr(b, lo, hi):
            """Query-reduce: two [1,512] PSUM groups (h-halves) per batch,
            each accumulating both column-subtiles of the (folded) tiles."""
            step = 2 if QFOLD else 1
            first = 1 if QFOLD else 0
            for i in range(lo + first, hi, step):
                t = qland[(b, i)]
                for half in range(2):
                    for c in range(2):
                        nc.tensor.matmul(
                            preps[b]["qr_ps"][half][:],
                            ones1[:],
                            t[:, c * H + half * A:c * H + (half + 1) * A],
                            start=(i == first and c == 0),
                            stop=(i == NQD - 1 and c == 1))

        def emit_prep(b):
            """qs -> qsT -> qp -> qpT -> u -> (w2 | wqbc) for batch b."""
            d = preps[b]
            qs_sb = sbr.tile([1, H], f32, tag="qs", bufs=1)
            for half in range(2):
                nc.scalar.copy(qs_sb[:, half * A:(half + 1) * A],
                               d["qr_ps"][half][:])
            qsT = sbr.tile([P, HJ], f32r, tag="qsT", bufs=1)
            if PREP_DMA:
                nc.scalar.dma_start(out=scr_qs[None, :], in_=qs_sb[:])
                qsT_st = sbr.tile([P, HJ], f32, tag="qsTst", bufs=1)
                nc.scalar.dma_start(
                    out=qsT_st[:],
                    in_=scr_qs.rearrange("(j p) -> p j", p=P))
                nc.scalar.copy(qsT[:], qsT_st[:])
            else:
                qsT_ps = ps_sm.tile([P, HJ], f32, tag="small")
                for j in range(HJ):
                    nc.tensor.matmul(qsT_ps[:, j:j + 1],
                                     qs_sb[:, j * P:(j + 1) * P], one11[:],
                                     start=True, stop=True)
                nc.scalar.copy(qsT[:], qsT_ps[:])
            qp_ps = ps_sm.tile([1, A], f32, tag="small")
            for j in range(HJ):
                nc.tensor.matmul(qp_ps[:], qsT[:, j:j + 1],
                                 Wq_r[:, j * A:(j + 1) * A],
                                 start=(j == 0), stop=(j == HJ - 1))
            qp_sb = sbr.tile([1, A], f32, tag="qp", bufs=1)
            nc.vector.scalar_tensor_tensor(
                out=qp_sb[:], in0=bq_row[:], scalar=float(LQ), in1=qp_ps[:],
                op0=mybir.AluOpType.mult, op1=mybir.AluOpType.add)
            qpT = sbr.tile([P, AC], f32r, tag="qpT", bufs=1)
            if PREP_DMA:
                nc.scalar.dma_start(out=scr_qp[None, :], in_=qp_sb[:])
                qpT_st = sbr.tile([P, AC], f32, tag="qpTst", bufs=1)
                nc.scalar.dma_start(
                    out=qpT_st[:],
                    in_=scr_qp.rearrange("(c p) -> p c", p=P))
                nc.scalar.copy(qpT[:], qpT_st[:])
            else:
                qpT_ps = ps_sm.tile([P, AC], f32, tag="small")
                for c in range(AC):
                    nc.tensor.matmul(qpT_ps[:, c:c + 1],
                                     qp_sb[:, c * P:(c + 1) * P], one11[:],
                                     start=True, stop=True)
                nc.scalar.copy(qpT[:], qpT_ps[:])
            u_sb = sbr.tile([1, H], f32, tag="u", bufs=1)
            for half in range(2):
                u_ps = ps_sm.tile([1, A], f32, tag="small")
                for c in range(AC):
                    nc.tensor.matmul(
                        u_ps[:], qpT[:, c:c + 1],
                        WkT_r[:, c * H + half * A:c * H + (half + 1) * A],
                        start=(c == 0), stop=(c == AC - 1))
                nc.scalar.copy(u_sb[:, half * A:(half + 1) * A], u_ps[:])
            if b == 3:
                w2 = sbr.tile([P, 2 * HJ], f32r, tag="w2")
                if PREP_DMA:
                    nc.scalar.dma_start(out=scr_u[None, :], in_=u_sb[:])
                    w2_st = sbr.tile([P, 2 * HJ], f32, tag="w2st")
                    nc.scalar.dma_start(
                        out=w2_st[:].rearrange("p (j two) -> p two j", two=2)
                        [:, 0:1, :],
                        in_=scr_u.rearrange("(j p) -> p j", p=P))
                    nc.scalar.copy(
                        w2_st[:].rearrange("p (j two) -> p j two", two=2)
                        [:, :, 1:2],
                        wT8[:].unsqueeze(2))
                    nc.scalar.copy(w2[:], w2_st[:])
                else:
                    w2_ps = ps_sm.tile([P, 2 * HJ], f32, tag="small")
                    for j in range(HJ):
                        nc.tensor.matmul(w2_ps[:, 2 * j:2 * j + 1],
                                         u_sb[:, j * P:(j + 1) * P], one11[:],
                                         start=True, stop=True)
                    nc.scalar.copy(w2[:], w2_ps[:])
                    nc.scalar.copy(
                        w2[:].rearrange("p (j two) -> p j two", two=2)[:, :, 1:2],
                        wT8[:].unsqueeze(2))
                d["w2"] = w2
            else:
                wqbc = sbr.tile([P, H], f32, tag=f"wqbc{b}")
                for half in range(2):
                    bc_ps = ps_sm.tile([P, A], f32, tag="small")
                    nc.tensor.matmul(bc_ps[:], ones_k1[:],
                                     u_sb[:, half * A:(half + 1) * A],
                                     start=True, stop=True)
                    nc.scalar.copy(wqbc[:, half * A:(half + 1) * A], bc_ps[:])
                d["wqbc"] = wqbc

        def emit_route_pe(g):
            """b3 granule g: transposes -> keyT (f32r) -> s2 matmuls -> sv."""
            t = kt[(3, g)]
            keyT = sbk.tile([P, 2 * H], f32r, tag="keyT", bufs=KEYT_BUFS)
            for j in range(HJ):
                ktp = ps_kt.tile([P, 2 * P], f32, tag="ktp")
                for c in range(2):
                    nc.tensor.transpose(ktp[:, c * P:(c + 1) * P],
                                        t[:, c * H + j * P:c * H + (j + 1) * P],
                                        ident[:])
                nc.scalar.copy(keyT[:, j * 2 * P:(j + 1) * 2 * P], ktp[:])
            s2 = ps_s2.tile([2, 2 * P], f32, tag="s2")
            w2 = preps[3]["w2"]
            for j in range(HJ):
                nc.tensor.matmul(s2[:], w2[:, 2 * j:2 * j + 2],
                                 keyT[:, j * 2 * P:(j + 1) * 2 * P],
                                 start=(j == 0), stop=(j == HJ - 1))
            nc.scalar.copy(sv3[:, g * GR:(g + 1) * GR], s2[:])

        def emit_dots_dve(b, g):
            t = kt[(b, g)]
            for c in range(2):
                ti = g * 2 + c
                j1 = sbj.tile([P, H], f32, tag="junk", bufs=2)
                nc.vector.scalar_tensor_tensor(
                    out=j1[:], in0=t[:, c * H:(c + 1) * H], scalar=1.0,
                    in1=preps[b]["wqbc"][:], op0=mybir.AluOpType.mult,
                    op1=mybir.AluOpType.mult, accum_out=sdve[b][:, ti:ti + 1])
                j2 = sbj.tile([P, H], f32, tag="junk", bufs=2)
                nc.vector.scalar_tensor_tensor(
                    out=j2[:], in0=t[:, c * H:(c + 1) * H], scalar=1.0,
                    in1=wvbc[:], op0=mybir.AluOpType.mult,
                    op1=mybir.AluOpType.mult, accum_out=vdve[b][:, ti:ti + 1])

        def emit_softmax_pe():
            vsw = sbj.tile([2, LK], f32, tag="vsw")
            nc.vector.stream_shuffle(vsw[:], sv3[:], [1, 0] + list(range(2, 32)))
            smax = sbsm.tile([2, 1], f32, tag="smax")
            nc.vector.reduce_max(smax[:], sv3[:], axis=mybir.AxisListType.X)
            nmax = sbsm.tile([2, 1], f32, tag="nmax")
            nc.vector.tensor_scalar_mul(nmax[:], smax[:], -1.0)
            den = sbsm.tile([1, 1], f32, tag="den")
            nc.scalar.activation(sv3[0:1, :], sv3[0:1, :],
                                 mybir.ActivationFunctionType.Exp,
                                 bias=nmax[0:1], scale=1.0, accum_out=den[:])
            num = sbsm.tile([1, 1], f32, tag="num")
            nc.vector.scalar_tensor_tensor(
                out=vsw[0:1, :], in0=sv3[0:1, :], scalar=1.0, in1=vsw[0:1, :],
                op0=mybir.AluOpType.mult, op1=mybir.AluOpType.mult,
                accum_out=num[:])
            _emit_final(nc, sbsm, num, den, cv_sb, out, 3)

        def emit_softmax_dve(b):
            m1 = sbsm.tile([P, 1], f32, tag="m1")
            nc.vector.reduce_max(m1[:], sdve[b][:], axis=mybir.AxisListType.X)
            mT_ps = ps_sm.tile([1, P], f32, tag="small")
            nc.tensor.transpose(mT_ps[:], m1[:], ident[:])
            mT_sb = sbsm.tile([1, P], f32, tag="mT")
            nc.vector.tensor_copy(mT_sb[:], mT_ps[:])
            gmax = sbsm.tile([1, 1], f32, tag="gmax")
            nc.vector.reduce_max(gmax[:], mT_sb[:], axis=mybir.AxisListType.X)
            ng_ps = ps_sm.tile([P, 1], f32, tag="small")
            nc.tensor.matmul(ng_ps[:], ones_k1[:], gmax[:], start=True, stop=True)
            ngm = sbsm.tile([P, 1], f32, tag="ngm")
            nc.vector.tensor_scalar_mul(ngm[:], ng_ps[:], -1.0)
            e128 = sbsm.tile([P, 2 * NG], f32, tag="e128")
            erow = sbsm.tile([P, 1], f32, tag="erowp")
            nc.scalar.activation(e128[:], sdve[b][:],
                                 mybir.ActivationFunctionType.Exp,
                                 bias=ngm[:], scale=1.0, accum_out=erow[:])
            junk5 = sbsm.tile([P, 2 * NG], f32, tag="junk5")
            nrow = sbsm.tile([P, 1], f32, tag="nrow")
            nc.vector.scalar_tensor_tensor(
                out=junk5[:], in0=e128[:], scalar=1.0, in1=vdve[b][:],
                op0=mybir.AluOpType.mult, op1=mybir.AluOpType.mult,
                accum_out=nrow[:])
            den_ps = ps_sm.tile([1, 2], f32, tag="small")
            nc.tensor.matmul(den_ps[:, 0:1], erow[:], ones1[:], start=True, stop=True)
            nc.tensor.matmul(den_ps[:, 1:2], nrow[:], ones1[:], start=True, stop=True)
            dn = sbsm.tile([1, 2], f32, tag="dn")
            nc.vector.tensor_copy(dn[:], den_ps[:])
            _emit_final(nc, sbsm, dn[:, 1:2], dn[:, 0:1], cv_sb, out, b)

        # ---------------- main emission interleave ----------------
        # Call order defines each engine's in-order stream; comments give the
        # intended wall-clock position.
        emit_qdma(3, 0, NQD)                 # q3 streams [0, ~28us]
        if QFOLD:
            emit_qfold(3, 0, NQD)
        emit_qr(3, 0, NQD)
        emit_prep(3)                         # ~30us
        emit_route_pe(0)
        emit_qdma(0, 0, 4)
        emit_route_pe(1)
        emit_qdma(0, 4, NQD)
        if QFOLD:
            emit_qfold(0, 0, NQD)
        emit_qr(0, 0, NQD)
        emit_prep(0)                         # ~58us
        emit_dots_dve(0, 0)
        emit_route_pe(2)
        emit_qdma(1, 0, 4)
        emit_dots_dve(0, 1)
        emit_route_pe(3)
        emit_qdma(1, 4, NQD)
        if QFOLD:
            emit_qfold(1, 0, NQD)
        emit_qr(1, 0, NQD)
        emit_prep(1)                         # ~85us
        emit_dots_dve(1, 0)
        emit_dots_dve(1, 1)
        emit_route_pe(4)
        emit_qdma(2, 0, 4)
        emit_kdma2(0, 3)                     # b2 early granules (ring-bounded)
        emit_dots_dve(0, 2)
        emit_dots_dve(1, 2)
        emit_route_pe(5)
        emit_qdma(2, 4, NQD)
        if QFOLD:
            emit_qfold(2, 0, NQD)
        emit_qr(2, 0, NQD)
        emit_prep(2)                         # ~110us
        emit_kdma2(3, NG)                    # rest of b2 (after prep2 copies)
        emit_dots_dve(2, 0)
        emit_dots_dve(2, 1)
        emit_dots_dve(2, 2)
        emit_route_pe(6)
        for g in range(3, NG):
            emit_dots_dve(0, g)
            emit_dots_dve(1, g)
            emit_dots_dve(2, g)
        emit_route_pe(7)
        emit_softmax_pe()
        for b in range(3):
            emit_softmax_dve(b)


def _emit_final(nc, sbsm, num, den, cv_sb, out, b):
    rden = sbsm.tile([1, 1], f32, tag="rden")
    nc.vector.reciprocal(rden[:], den[:])
    x = sbsm.tile([1, 1], f32, tag="x")
    nc.vector.tensor_tensor(out=x[:], in0=num[:], in1=rden[:],
                            op=mybir.AluOpType.mult)
    x2 = sbsm.tile([1, 1], f32, tag="x2")
    nc.vector.tensor_tensor(out=x2[:], in0=x[:], in1=cv_sb[:],
                            op=mybir.AluOpType.add)
    nc.sync.dma_start(out=out[b:b + 1, :], in_=x2[:])


def _shard(query, key, shared):
    in_maps = []
    for c in range(N_CORES):
        sl = slice(c * BPC, (c + 1) * BPC)
        m = {"query": np.ascontiguousarray(query[sl]),
             "key": np.ascontiguousarray(key[sl])}
        m.update(shared)
        in_maps.append(m)
    return in_maps


def _make_in_maps(inputs):
    query = np.ascontiguousarray(np.asarray(inputs["query"], dtype=np.float32))
    key = np.ascontiguousarray(np.asarray(inputs["key"], dtype=np.float32))
    shared = {k: np.ascontiguousarray(np.asarray(inputs[k], dtype=np.float32))
              for k in ("Wq", "bq", "Wk", "bk", "Wv", "bv")}
    return _shard(query, key, shared)


def kernel(**inputs):
    if "nc" not in _CACHE:
        _CACHE["nc"] = build_bass()
    nc = _CACHE["nc"]
    in_maps = _make_in_maps(inputs)
    res = run_bass_kernel_spmd(nc, in_maps, list(range(N_CORES)))
    outs = [res.results[c]["out"] for c in range(N_CORES)]
    return np.concatenate(outs, axis=0).astype(np.float32)


if __name__ == "__main__":
    rng = np.random.default_rng(0)
    ins = {
        "query": rng.standard_normal((B, LQ, H), dtype=np.float32),
        "key": rng.standard_normal((B, LK, H), dtype=np.float32),
        "Wq": (rng.standard_normal((H, A), dtype=np.float32) / np.sqrt(H)).astype(np.float32),
        "bq": np.zeros((A,), np.float32),
        "Wk": (rng.standard_normal((H, A), dtype=np.float32) / np.sqrt(H)).astype(np.float32),
        "bk": np.zeros((A,), np.float32),
        "Wv": (rng.standard_normal((A, 1), dtype=np.float32) / np.sqrt(A)).astype(np.float32),
        "bv": np.zeros((1,), np.float32),
    }
    x = kernel(**ins)
    print("kernel out:", x[:4, 0])


# revision 12
# speedup vs baseline: 1.0893x; 1.0893x over previous
"""Trainium2 Bass kernel for nn_Attention_2 (B=32, LQ=LK=2048, H=1024, A=512).

Math (q-sum distributes through the matmul, so [B,LQ,LK] never exists):
  qs[b]   = sum_q query[b,q,:]
  qp[b]   = qs[b] @ Wq + LQ*bq
  u[b]    = qp[b] @ Wk^T            (score weights, [H])
  w       = Wk @ Wv[:,0]            (v weights, [H])
  s[b,k]  = key[b,k,:] . u[b]  (+ qp.bk const, cancels in softmax)
  v[b,k]  = key[b,k,:] . w     (+ cv = bk.Wv + bv, folded at end)
  x[b]    = softmax(s) . v + cv

v7 architecture (vs the DMA-accumulate baseline, ~367us):
  - All DMA is plain packets on the two hardware queues.  One hw queue
    sustains ~378 GB/s (measured); accumulate packets run the SDMA engines
    at half rate, so the query reduction moved on-chip (PE ones-matmuls).
  - Query streams batch-sequentially (order b3,b0,b1,b2) on the ACT queue;
    each batch's prep (qs->qsT->qp->qpT->u->w2/wqbc) runs as soon as its
    last tile lands, so that batch's key dots start while later batches'
    query still streams.
  - Key streams in batch-interleaved granule waves (256 rows / 1MB):
    b3/b0/b1 on the SYNC queue; b2 on the ACT queue split around prep2 so
    a ring-full dma_start can never park the ACT stream ahead of the
    copies that would free it (deadlock audit).
  - b3 takes the PE route: fp32 transposes -> ACT copy (rounds to f32r)
    -> f32r matmuls with the [uT|wT] stationary pair (f32r = 1 cyc/col
    for moving dims >=256 vs 4 for fp32; 11 mantissa bits -> measured
    end-to-end rel-err ~2.6e-3, gate 2e-2).  b0..b2 take the DVE route
    (fused mul+reduce STT rowwise dots, fp32).
  - Raw Wk/Wq stage through the DVE key ring (consumed in the first ~15us)
    so only the rounded/transposed copies hold permanent SBUF.
"""
import numpy as np

import concourse.bass as bass
import concourse.bacc as bacc
import concourse.tile as tile
from concourse import mybir
from concourse.bass_utils import run_bass_kernel_spmd

N_CORES = 8
B, LQ, LK, H, A = 32, 2048, 2048, 1024, 512
BPC = B // N_CORES
P = 128
f32 = mybir.dt.float32
f32r = mybir.dt.float32r
NG = 8                      # key granules per batch (256 rows each)
GR = LK // NG               # 256
NQD = 8                     # query DMAs per batch ([128, 2H] tiles)
HJ = H // P                 # 8
AC = A // P                 # 4

_CACHE = {}
import os as _os
QLAND_BUFS = int(_os.environ.get("QLAND_BUFS", "3"))
KTPE_BUFS = int(_os.environ.get("KTPE_BUFS", "3"))
KTD01_BUFS = int(_os.environ.get("KTD01_BUFS", "4"))
KTD2_BUFS = int(_os.environ.get("KTD2_BUFS", "3"))
KEYT_BUFS = int(_os.environ.get("KEYT_BUFS", "2"))
QFOLD = int(_os.environ.get("QFOLD", "1"))
PREP_DMA = int(_os.environ.get("PREP_DMA", "0"))


def build_bass():
    nc = bacc.Bacc(None, target_bir_lowering=False, debug=False)

    query = nc.dram_tensor("query", [BPC, LQ, H], f32, kind="ExternalInput").ap()
    key = nc.dram_tensor("key", [BPC, LK, H], f32, kind="ExternalInput").ap()
    Wq = nc.dram_tensor("Wq", [H, A], f32, kind="ExternalInput").ap()
    bq = nc.dram_tensor("bq", [A], f32, kind="ExternalInput").ap()
    Wk = nc.dram_tensor("Wk", [H, A], f32, kind="ExternalInput").ap()
    bk = nc.dram_tensor("bk", [A], f32, kind="ExternalInput").ap()
    Wv = nc.dram_tensor("Wv", [A, 1], f32, kind="ExternalInput").ap()
    bv = nc.dram_tensor("bv", [1], f32, kind="ExternalInput").ap()
    out = nc.dram_tensor("out", [BPC, 1], f32, kind="ExternalOutput").ap()
    scr_qs = nc.dram_tensor("scr_qs", [H], f32, kind="Internal").ap()
    scr_qp = nc.dram_tensor("scr_qp", [A], f32, kind="Internal").ap()
    scr_u = nc.dram_tensor("scr_u", [H], f32, kind="Internal").ap()

    with tile.TileContext(nc) as tc:
        _build_body(nc, tc, query, key, Wq, bq, Wk, bk, Wv, bv, out,
                    scr_qs, scr_qp, scr_u)
    nc.compile()
    return nc


def _build_body(nc, tc, query, key, Wq, bq, Wk, bk, Wv, bv, out,
                scr_qs, scr_qp, scr_u):
    from contextlib import ExitStack
    ctx = ExitStack()
    with ctx:
        sbc = ctx.enter_context(tc.tile_pool(name="sbc", bufs=1))
        sbq = ctx.enter_context(tc.tile_pool(name="sbq", bufs=1))
        sbk = ctx.enter_context(tc.tile_pool(name="sbk", bufs=1))
        sbr = ctx.enter_context(tc.tile_pool(name="sbr", bufs=1))
        sbsv = ctx.enter_context(tc.tile_pool(name="sbsv", bufs=1))
        sbj = ctx.enter_context(tc.tile_pool(name="sbj", bufs=1))
        sbsm = ctx.enter_context(tc.tile_pool(name="sbsm", bufs=1))
        ps_qr = ctx.enter_context(tc.tile_pool(name="ps_qr", bufs=1, space="PSUM"))
        ps_kt = ctx.enter_context(tc.tile_pool(name="ps_kt", bufs=2, space="PSUM"))
        ps_s2 = ctx.enter_context(tc.tile_pool(name="ps_s2", bufs=2, space="PSUM"))
        ps_sm = ctx.enter_context(tc.tile_pool(name="ps_sm", bufs=1, space="PSUM"))

        # ---------------- constants ----------------
        ident = sbc.tile([P, P], f32)
        colidx = sbsm.tile([P, P], f32, tag="small")
        rowidx = sbsm.tile([P, 1], f32, tag="tiny")
        nc.gpsimd.iota(colidx[:], pattern=[[1, P]], base=0, channel_multiplier=0,
                       allow_small_or_imprecise_dtypes=True)
        nc.gpsimd.iota(rowidx[:], pattern=[[0, 1]], base=0, channel_multiplier=1,
                       allow_small_or_imprecise_dtypes=True)
        nc.vector.tensor_scalar(out=ident[:], in0=colidx[:], scalar1=rowidx[:],
                                scalar2=None, op0=mybir.AluOpType.is_equal)
        ones1 = sbc.tile([P, 1], f32)
        nc.vector.memset(ones1[:], 1.0)
        one11 = sbc.tile([1, 1], f32)
        nc.vector.memset(one11[:], 1.0)
        ones_k1 = sbc.tile([1, P], f32)
        nc.vector.memset(ones_k1[:], 1.0)

        # -------- weight DMAs (ACT queue, first) --------
        # raw Wk/Wq stage through the ktd01 ring (slots recycled for keys)
        wk_st = [sbk.tile([P, 4 * A], f32, tag="ktd01", bufs=KTD01_BUFS,
                          name=f"wkst{i}") for i in range(2)]
        for i in range(2):
            nc.scalar.dma_start(
                out=wk_st[i][:].rearrange("p (j a) -> p j a", j=4),
                in_=Wk[i * A:(i + 1) * A, :].rearrange("(j p) a -> p j a", p=P))
        wq_st = [sbk.tile([P, 4 * A], f32, tag="ktd01", bufs=KTD01_BUFS,
                          name=f"wqst{i}") for i in range(2)]
        for i in range(2):
            nc.scalar.dma_start(
                out=wq_st[i][:].rearrange("p (j a) -> p j a", j=4),
                in_=Wq[i * A:(i + 1) * A, :].rearrange("(j p) a -> p j a", p=P))
        wv_sb = sbc.tile([P, AC], f32)
        nc.scalar.dma_start(out=wv_sb[:].rearrange("p (c o) -> p c o", c=AC),
                            in_=Wv.rearrange("(c p) o -> p c o", p=P))
        bk_sb = sbc.tile([P, AC], f32)
        nc.scalar.dma_start(out=bk_sb[:], in_=bk.rearrange("(c p) -> p c", p=P))
        bv_sb = sbc.tile([1, 1], f32)
        nc.scalar.dma_start(out=bv_sb[:], in_=bv[None, :])
        bq_row = sbc.tile([1, A], f32)
        nc.scalar.dma_start(out=bq_row[:], in_=bq[None, :])

        # -------- one-time weight prep (PE+ACT+DVE, before query issues) ----
        # WkT_r [a-part, (c h)] f32r, rounded at the PSUM->SBUF copies
        WkT_r = sbc.tile([P, AC * H], f32r)
        for c in range(AC):
            for half in range(2):
                wkp = ps_kt.tile([P, A], f32, tag="ktp")
                for jl in range(4):
                    nc.tensor.transpose(
                        wkp[:, jl * P:(jl + 1) * P],
                        wk_st[half][:, jl * A + c * P:jl * A + (c + 1) * P],
                        ident[:])
                nc.scalar.copy(
                    WkT_r[:, c * H + half * A:c * H + (half + 1) * A], wkp[:])
        # Wq_r [h-part, (j a)] f32r
        Wq_r = sbc.tile([P, HJ * A], f32r)
        for i in range(2):
            nc.scalar.copy(Wq_r[:, i * 4 * A:(i + 1) * 4 * A], wq_st[i][:])
        # wv_r, then w row = Wv^T @ WkT  [1, H] fp32
        wv_r = sbc.tile([P, AC], f32r)
        nc.scalar.copy(wv_r[:], wv_sb[:])
        w_sb = sbr.tile([1, H], f32, tag="u", bufs=1, name="w_sb")
        for half in range(2):
            w_ps = ps_sm.tile([1, A], f32, tag="small")
            for c in range(AC):
                nc.tensor.matmul(w_ps[:], wv_r[:, c:c + 1],
                                 WkT_r[:, c * H + half * A:c * H + (half + 1) * A],
                                 start=(c == 0), stop=(c == AC - 1))
            nc.scalar.copy(w_sb[:, half * A:(half + 1) * A], w_ps[:])
        # wvbc [P, H] = broadcast w across partitions (DVE route operand)
        wvbc = sbc.tile([P, H], f32)
        for half in range(2):
            bc_ps = ps_sm.tile([P, A], f32, tag="small")
            nc.tensor.matmul(bc_ps[:], ones_k1[:], w_sb[:, half * A:(half + 1) * A],
                             start=True, stop=True)
            nc.scalar.copy(wvbc[:, half * A:(half + 1) * A], bc_ps[:])
        # wT8 [128, 8]: w chunks as columns (for the b3 w2 tile)
        wT8_ps = ps_sm.tile([P, HJ], f32, tag="small")
        for j in range(HJ):
            nc.tensor.matmul(wT8_ps[:, j:j + 1], w_sb[:, j * P:(j + 1) * P],
                             one11[:], start=True, stop=True)
        wT8 = sbc.tile([P, HJ], f32)
        nc.scalar.copy(wT8[:], wT8_ps[:])
        # cv = bk . Wv + bv
        junk4 = sbsm.tile([P, AC], f32, tag="tiny2")
        cvcol = sbsm.tile([P, 1], f32, tag="tiny3")
        nc.vector.scalar_tensor_tensor(out=junk4[:], in0=bk_sb[:], scalar=1.0,
                                       in1=wv_sb[:], op0=mybir.AluOpType.mult,
                                       op1=mybir.AluOpType.mult, accum_out=cvcol[:])
        cv_ps = ps_sm.tile([1, 1], f32, tag="small")
        nc.tensor.matmul(cv_ps[:], cvcol[:], ones1[:], start=True, stop=True)
        cv_sb = sbc.tile([1, 1], f32)
        nc.vector.tensor_tensor(out=cv_sb[:], in0=cv_ps[:], in1=bv_sb[:],
                                op=mybir.AluOpType.add)

        # -------- key DMAs for b3/b0/b1 (SYNC queue, granule waves) --------
        kt = {}
        for g in range(NG):
            for b in (3, 0, 1):
                tag, bufs = ("ktpe", KTPE_BUFS) if b == 3 else ("ktd01", KTD01_BUFS)
                t = sbk.tile([P, 2 * H], f32, tag=tag, bufs=bufs, name=f"k{b}_{g}")
                nc.sync.dma_start(
                    out=t[:].rearrange("p (c h) -> p c h", c=2),
                    in_=key[b, g * GR:(g + 1) * GR, :]
                    .rearrange("(c p) h -> p c h", p=P))
                kt[(b, g)] = t

        # ---------------- per-batch state ----------------
        preps = {b: {} for b in range(BPC)}
        sv3 = sbsv.tile([2, LK], f32, tag="sv3")
        sdve = {b: sbsv.tile([P, 2 * NG], f32, tag=f"sd{b}", name=f"sdve{b}") for b in range(3)}
        vdve = {b: sbsv.tile([P, 2 * NG], f32, tag=f"vd{b}", name=f"vdve{b}") for b in range(3)}
        qland = {}

        def emit_qdma(b, lo, hi):
            if lo == 0:
                preps[b]["qr_ps"] = [ps_qr.tile([1, A], f32, tag=f"qr{h}", bufs=1,
                                                 name=f"qrps{b}_{h}")
                                     for h in range(2)]
            for i in range(lo, hi):
                t = sbq.tile([P, 2 * H], f32, tag="qland", bufs=QLAND_BUFS,
                             name=f"q{b}_{i}")
                nc.scalar.dma_start(
                    out=t[:].rearrange("p (c h) -> p c h", c=2),
                    in_=query[b, i * 2 * P:(i + 1) * 2 * P, :]
                    .rearrange("(c p) h -> p c h", p=P))
                qland[(b, i)] = t

        def emit_kdma2(glo, ghi):
            """b2 key granules on the ACT queue (split around prep2)."""
            for g in range(glo, ghi):
                t = sbk.tile([P, 2 * H], f32, tag="ktd2", bufs=KTD2_BUFS,
                             name=f"k2_{g}")
                nc.scalar.dma_start(
                    out=t[:].rearrange("p (c h) -> p c h", c=2),
                    in_=key[2, g * GR:(g + 1) * GR, :]
                    .rearrange("(c p) h -> p c h", p=P))
                kt[(2, g)] = t

        def emit_qfold(b, lo, hi):
            """GpSimd pair-folds: tile 2i+1 += tile 2i (in place)."""
            for i in range(lo, hi, 2):
                t0, t1 = qland[(b, i)], qland[(b, i + 1)]
                nc.gpsimd.tensor_tensor(out=t1[:], in0=t0[:], in1=t1[:],
                                        op=mybir.AluOpType.add)

        def emit_qr(b, lo, hi):
            """Query-reduce: two [1,512] PSUM groups (h-halves) per batch,
            each accumulating both column-subtiles of the (folded) tiles."""
            step = 2 if QFOLD else 1
            first = 1 if QFOLD else 0
            for i in range(lo + first, hi, step):
                t = qland[(b, i)]
                for half in range(2):
                    for c in range(2):
                        nc.tensor.matmul(
                            preps[b]["qr_ps"][half][:],
                            ones1[:],
                            t[:, c * H + half * A:c * H + (half + 1) * A],
                            start=(i == first and c == 0),
                            stop=(i == NQD - 1 and c == 1))

        def emit_prep(b):
            """qs -> qsT -> qp -> qpT -> u -> (w2 | wqbc) for batch b."""
            d = preps[b]
            qs_sb = sbr.tile([1, H], f32, tag="qs", bufs=1)
            for half in range(2):
                nc.scalar.copy(qs_sb[:, half * A:(half + 1) * A],
                               d["qr_ps"][half][:])
            qsT = sbr.tile([P, HJ], f32r, tag="qsT", bufs=1)
            if PREP_DMA:
                nc.scalar.dma_start(out=scr_qs[None, :], in_=qs_sb[:])
                qsT_st = sbr.tile([P, HJ], f32, tag="qsTst", bufs=1)
                nc.scalar.dma_start(
                    out=qsT_st[:],
                    in_=scr_qs.rearrange("(j p) -> p j", p=P))
                nc.scalar.copy(qsT[:], qsT_st[:])
            else:
                qsT_ps = ps_sm.tile([P, HJ], f32, tag="small")
                for j in range(HJ):
                    nc.tensor.matmul(qsT_ps[:, j:j + 1],
                                     qs_sb[:, j * P:(j + 1) * P], one11[:],
                                     start=True, stop=True)
                nc.scalar.copy(qsT[:], qsT_ps[:])
            qp_ps = ps_sm.tile([1, A], f32, tag="small")
            for j in range(HJ):
                nc.tensor.matmul(qp_ps[:], qsT[:, j:j + 1],
                                 Wq_r[:, j * A:(j + 1) * A],
                                 start=(j == 0), stop=(j == HJ - 1))
            qp_sb = sbr.tile([1, A], f32, tag="qp", bufs=1)
            nc.vector.scalar_tensor_tensor(
                out=qp_sb[:], in0=bq_row[:], scalar=float(LQ), in1=qp_ps[:],
                op0=mybir.AluOpType.mult, op1=mybir.AluOpType.add)
            qpT = sbr.tile([P, AC], f32r, tag="qpT", bufs=1)
            if PREP_DMA:
                nc.scalar.dma_start(out=scr_qp[None, :], in_=qp_sb[:])
                qpT_st = sbr.tile([P, AC], f32, tag="qpTst", bufs=1)
                nc.scalar.dma_start(
                    out=qpT_st[:],
                    in_=scr_qp.rearrange("(c p) -> p c", p=P))
                nc.scalar.copy(qpT[:], qpT_st[:])
            else:
                qpT_ps = ps_sm.tile([P, AC], f32, tag="small")
                for c in range(AC):
                    nc.tensor.matmul(qpT_ps[:, c:c + 1],
                                     qp_sb[:, c * P:(c + 1) * P], one11[:],
                                     start=True, stop=True)
                nc.scalar.copy(qpT[:], qpT_ps[:])
            u_sb = sbr.tile([1, H], f32, tag="u", bufs=1)
            for half in range(2):
                u_ps = ps_sm.tile([1, A], f32, tag="small")
                for c in range(AC):
                    nc.tensor.matmul(
                        u_ps[:], qpT[:, c:c + 1],
                        WkT_r[:, c * H + half * A:c * H + (half + 1) * A],
                        start=(c == 0), stop=(c == AC - 1))
                nc.scalar.copy(u_sb[:, half * A:(half + 1) * A], u_ps[:])
            if b == 3:
                w2 = sbr.tile([P, 2 * HJ], f32r, tag="w2")
                if PREP_DMA:
                    nc.scalar.dma_start(out=scr_u[None, :], in_=u_sb[:])
                    w2_st = sbr.tile([P, 2 * HJ], f32, tag="w2st")
                    nc.scalar.dma_start(
                        out=w2_st[:].rearrange("p (j two) -> p two j", two=2)
                        [:, 0:1, :],
                        in_=scr_u.rearrange("(j p) -> p j", p=P))
                    nc.scalar.copy(
                        w2_st[:].rearrange("p (j two) -> p j two", two=2)
                        [:, :, 1:2],
                        wT8[:].unsqueeze(2))
                    nc.scalar.copy(w2[:], w2_st[:])
                else:
                    w2_ps = ps_sm.tile([P, 2 * HJ], f32, tag="small")
                    for j in range(HJ):
                        nc.tensor.matmul(w2_ps[:, 2 * j:2 * j + 1],
                                         u_sb[:, j * P:(j + 1) * P], one11[:],
                                         start=True, stop=True)
                    nc.scalar.copy(w2[:], w2_ps[:])
                    nc.scalar.copy(
                        w2[:].rearrange("p (j two) -> p j two", two=2)[:, :, 1:2],
                        wT8[:].unsqueeze(2))
                d["w2"] = w2
            else:
                wqbc = sbr.tile([P, H], f32, tag=f"wqbc{b}")
                for half in range(2):
                    bc_ps = ps_sm.tile([P, A], f32, tag="small")
                    nc.tensor.matmul(bc_ps[:], ones_k1[:],
                                     u_sb[:, half * A:(half + 1) * A],
                                     start=True, stop=True)
                    nc.scalar.copy(wqbc[:, half * A:(half + 1) * A], bc_ps[:])
                d["wqbc"] = wqbc

        def emit_route_pe(g):
            """b3 granule g: transposes -> keyT (f32r) -> s2 matmuls -> sv."""
            t = kt[(3, g)]
            keyT = sbk.tile([P, 2 * H], f32r, tag="keyT", bufs=KEYT_BUFS)
            for j in range(HJ):
                ktp = ps_kt.tile([P, 2 * P], f32, tag="ktp")
                for c in range(2):
                    nc.tensor.transpose(ktp[:, c * P:(c + 1) * P],
                                        t[:, c * H + j * P:c * H + (j + 1) * P],
                                        ident[:])
                nc.scalar.copy(keyT[:, j * 2 * P:(j + 1) * 2 * P], ktp[:])
            s2 = ps_s2.tile([2, 2 * P], f32, tag="s2")
            w2 = preps[3]["w2"]
            for j in range(HJ):
                nc.tensor.matmul(s2[:], w2[:, 2 * j:2 * j + 2],
                                 keyT[:, j * 2 * P:(j + 1) * 2 * P],
                                 start=(j == 0), stop=(j == HJ - 1))
            nc.scalar.copy(sv3[:, g * GR:(g + 1) * GR], s2[:])

        def emit_dots_dve(b, g):
            t = kt[(b, g)]
            for c in range(2):
                ti = g * 2 + c
                j1 = sbj.tile([P, H], f32, tag="junk", bufs=2)
                nc.vector.scalar_tensor_tensor(
                    out=j1[:], in0=t[:, c * H:(c + 1) * H], scalar=1.0,
                    in1=preps[b]["wqbc"][:], op0=mybir.AluOpType.mult,
                    op1=mybir.AluOpType.mult, accum_out=sdve[b][:, ti:ti + 1])
                j2 = sbj.tile([P, H], f32, tag="junk", bufs=2)
                nc.vector.scalar_tensor_tensor(
                    out=j2[:], in0=t[:, c * H:(c + 1) * H], scalar=1.0,
                    in1=wvbc[:], op0=mybir.AluOpType.mult,
                    op1=mybir.AluOpType.mult, accum_out=vdve[b][:, ti:ti + 1])

        def emit_softmax_pe():
            vsw = sbj.tile([2, LK], f32, tag="vsw")
            nc.vector.stream_shuffle(vsw[:], sv3[:], [1, 0] + list(range(2, 32)))
            smax = sbsm.tile([2, 1], f32, tag="smax")
            nc.vector.reduce_max(smax[:], sv3[:], axis=mybir.AxisListType.X)
            nmax = sbsm.tile([2, 1], f32, tag="nmax")
            nc.vector.tensor_scalar_mul(nmax[:], smax[:], -1.0)
            den = sbsm.tile([1, 1], f32, tag="den")
            nc.scalar.activation(sv3[0:1, :], sv3[0:1, :],
                                 mybir.ActivationFunctionType.Exp,
                                 bias=nmax[0:1], scale=1.0, accum_out=den[:])
            num = sbsm.tile([1, 1], f32, tag="num")
            nc.vector.scalar_tensor_tensor(
                out=vsw[0:1, :], in0=sv3[0:1, :], scalar=1.0, in1=vsw[0:1, :],
                op0=mybir.AluOpType.mult, op1=mybir.AluOpType.mult,
                accum_out=num[:])
            _emit_final(nc, sbsm, num, den, cv_sb, out, 3)

        def emit_softmax_dve(b):
            m1 = sbsm.tile([P, 1], f32, tag="m1")
            nc.vector.reduce_max(m1[:], sdve[b][:], axis=mybir.AxisListType.X)
            mT_ps = ps_sm.tile([1, P], f32, tag="small")
            nc.tensor.transpose(mT_ps[:], m1[:], ident[:])
            mT_sb = sbsm.tile([1, P], f32, tag="mT")
            nc.vector.tensor_copy(mT_sb[:], mT_ps[:])
            gmax = sbsm.tile([1, 1], f32, tag="gmax")
            nc.vector.reduce_max(gmax[:], mT_sb[:], axis=mybir.AxisListType.X)
            ng_ps = ps_sm.tile([P, 1], f32, tag="small")
            nc.tensor.matmul(ng_ps[:], ones_k1[:], gmax[:], start=True, stop=True)
            ngm = sbsm.tile([P, 1], f32, tag="ngm")
            nc.vector.tensor_scalar_mul(ngm[:], ng_ps[:], -1.0)
            e128 = sbsm.tile([P, 2 * NG], f32, tag="e128")
            erow = sbsm.tile([P, 1], f32, tag="erowp")
            nc.scalar.activation(e128[:], sdve[b][:],
                                 mybir.ActivationFunctionType.Exp,
                                 bias=ngm[:], scale=1.0, accum_out=erow[:])
            junk5 = sbsm.tile([P, 2 * NG], f32, tag="junk5")
            nrow = sbsm.tile([P, 1], f32, tag="nrow")
            nc.vector.scalar_tensor_tensor(
                out=junk5[:], in0=e128[:], scalar=1.0, in1=vdve[b][:],
                op0=mybir.AluOpType.mult, op1=mybir.AluOpType.mult,
                accum_out=nrow[:])
            den_ps = ps_sm.tile([1, 2], f32, tag="small")
            nc.tensor.matmul(den_ps[:, 0:1], erow[:], ones1[:], start=True, stop=True)
            nc.tensor.matmul(den_ps[:, 1:2], nrow[:], ones1[:], start=True, stop=True)
            dn = sbsm.tile([1, 2], f32, tag="dn")
            nc.vector.tensor_copy(dn[:], den_ps[:])
            _emit_final(nc, sbsm, dn[:, 1:2], dn[:, 0:1], cv_sb, out, b)

        # ---------------- main emission interleave ----------------
        # Call order defines each engine's in-order stream; comments give the
        # intended wall-clock position.
        emit_qdma(3, 0, NQD)                 # q3 streams [0, ~28us]
        if QFOLD:
            emit_qfold(3, 0, NQD)
        emit_qr(3, 0, NQD)
        emit_prep(3)                         # ~30us
        emit_route_pe(0)
        emit_qdma(0, 0, 4)
        emit_route_pe(1)
        emit_qdma(0, 4, NQD)
        if QFOLD:
            emit_qfold(0, 0, NQD)
        emit_qr(0, 0, NQD)
        emit_prep(0)                         # ~58us
        emit_dots_dve(0, 0)
        emit_route_pe(2)
        emit_qdma(1, 0, 4)
        emit_dots_dve(0, 1)
        emit_route_pe(3)
        emit_qdma(1, 4, NQD)
        if QFOLD:
            emit_qfold(1, 0, NQD)
        emit_qr(1, 0, NQD)
        emit_prep(1)                         # ~85us
        emit_dots_dve(1, 0)
        emit_dots_dve(1, 1)
        emit_route_pe(4)
        emit_qdma(2, 0, 4)
        emit_kdma2(0, 3)                     # b2 early granules (ring-bounded)
        emit_dots_dve(0, 2)
        emit_dots_dve(1, 2)
        emit_route_pe(5)
        emit_qdma(2, 4, NQD)
        if QFOLD:
            emit_qfold(2, 0, NQD)
        emit_qr(2, 0, NQD)
        emit_prep(2)                         # ~110us
        emit_kdma2(3, NG)                    # rest of b2 (after prep2 copies)
        emit_dots_dve(2, 0)
        emit_dots_dve(2, 1)
        emit_dots_dve(2, 2)
        emit_route_pe(6)
        for g in range(3, NG):
            emit_dots_dve(0, g)
            emit_dots_dve(1, g)
            emit_dots_dve(2, g)
        emit_route_pe(7)
        emit_softmax_pe()
        for b in range(3):
            emit_softmax_dve(b)


def _emit_final(nc, sbsm, num, den, cv_sb, out, b):
    rden = sbsm.tile([1, 1], f32, tag="rden")
    nc.vector.reciprocal(rden[:], den[:])
    x = sbsm.tile([1, 1], f32, tag="x")
    nc.vector.tensor_tensor(out=x[:], in0=num[:], in1=rden[:],
                            op=mybir.AluOpType.mult)
    x2 = sbsm.tile([1, 1], f32, tag="x2")
    nc.vector.tensor_tensor(out=x2[:], in0=x[:], in1=cv_sb[:],
                            op=mybir.AluOpType.add)
    nc.sync.dma_start(out=out[b:b + 1, :], in_=x2[:])


def _shard(query, key, shared):
    in_maps = []
    for c in range(N_CORES):
        sl = slice(c * BPC, (c + 1) * BPC)
        m = {"query": np.ascontiguousarray(query[sl]),
             "key": np.ascontiguousarray(key[sl])}
        m.update(shared)
        in_maps.append(m)
    return in_maps


def _make_in_maps(inputs):
    query = np.ascontiguousarray(np.asarray(inputs["query"], dtype=np.float32))
    key = np.ascontiguousarray(np.asarray(inputs["key"], dtype=np.float32))
    shared = {k: np.ascontiguousarray(np.asarray(inputs[k], dtype=np.float32))
              for k in ("Wq", "bq", "Wk", "bk", "Wv", "bv")}
    return _shard(query, key, shared)


def kernel(**inputs):
    if "nc" not in _CACHE:
        _CACHE["nc"] = build_bass()
    nc = _CACHE["nc"]
    in_maps = _make_in_maps(inputs)
    res = run_bass_kernel_spmd(nc, in_maps, list(range(N_CORES)))
    outs = [res.results[c]["out"] for c in range(N_CORES)]
    return np.concatenate(outs, axis=0).astype(np.float32)


if __name__ == "__main__":
    rng = np.random.default_rng(0)
    ins = {
        "query": rng.standard_normal((B, LQ, H), dtype=np.float32),
        "key": rng.standard_normal((B, LK, H), dtype=np.float32),
        "Wq": (rng.standard_normal((H, A), dtype=np.float32) / np.sqrt(H)).astype(np.float32),
        "bq": np.zeros((A,), np.float32),
        "Wk": (rng.standard_normal((H, A), dtype=np.float32) / np.sqrt(H)).astype(np.float32),
        "bk": np.zeros((A,), np.float32),
        "Wv": (rng.standard_normal((A, 1), dtype=np.float32) / np.sqrt(A)).astype(np.float32),
        "bv": np.zeros((1,), np.float32),
    }
    x = kernel(**ins)
    print("kernel out:", x[:4, 0])


# revision 17
# speedup vs baseline: 1.1430x; 1.0494x over previous
"""Trainium2 Bass kernel for nn_Attention_2 (B=32, LQ=LK=2048, H=1024, A=512).

Math (q-sum distributes through the matmul, so [B,LQ,LK] never exists):
  qs[b]   = sum_q query[b,q,:]
  qp[b]   = qs[b] @ Wq + LQ*bq
  u[b]    = qp[b] @ Wk^T            (score weights, [H])
  w       = Wk @ Wv[:,0]            (v weights, [H])
  s[b,k]  = key[b,k,:] . u[b]  (+ qp.bk const, cancels in softmax)
  v[b,k]  = key[b,k,:] . w     (+ cv = bk.Wv + bv, folded at end)
  x[b]    = softmax(s) . v + cv

v7 architecture (vs the DMA-accumulate baseline, ~367us):
  - All DMA is plain packets on the two hardware queues.  One hw queue
    sustains ~378 GB/s (measured); accumulate packets run the SDMA engines
    at half rate, so the query reduction moved on-chip (PE ones-matmuls).
  - Query streams batch-sequentially (order b3,b0,b1,b2) on the ACT queue;
    each batch's prep (qs->qsT->qp->qpT->u->w2/wqbc) runs as soon as its
    last tile lands, so that batch's key dots start while later batches'
    query still streams.
  - Key streams in batch-interleaved granule waves (256 rows / 1MB):
    b3/b0/b1 on the SYNC queue; b2 on the ACT queue split around prep2 so
    a ring-full dma_start can never park the ACT stream ahead of the
    copies that would free it (deadlock audit).
  - b3 takes the PE route: fp32 transposes -> ACT copy (rounds to f32r)
    -> f32r matmuls with the [uT|wT] stationary pair (f32r = 1 cyc/col
    for moving dims >=256 vs 4 for fp32; 11 mantissa bits -> measured
    end-to-end rel-err ~2.6e-3, gate 2e-2).  b0..b2 take the DVE route
    (fused mul+reduce STT rowwise dots, fp32).
  - Raw Wk/Wq stage through the DVE key ring (consumed in the first ~15us)
    so only the rounded/transposed copies hold permanent SBUF.
"""
import numpy as np

import concourse.bass as bass
import concourse.bacc as bacc
import concourse.tile as tile
from concourse import mybir
from concourse.bass_utils import run_bass_kernel_spmd

N_CORES = 8
B, LQ, LK, H, A = 32, 2048, 2048, 1024, 512
BPC = B // N_CORES
P = 128
f32 = mybir.dt.float32
f32r = mybir.dt.float32r
NG = 8                      # key granules per batch (256 rows each)
GR = LK // NG               # 256
NQD = 8                     # query DMAs per batch ([128, 2H] tiles)
HJ = H // P                 # 8
AC = A // P                 # 4

_CACHE = {}
import os as _os
QLAND_BUFS = int(_os.environ.get("QLAND_BUFS", "3"))
KTPE_BUFS = int(_os.environ.get("KTPE_BUFS", "3"))
KTD01_BUFS = int(_os.environ.get("KTD01_BUFS", "4"))
KTD2_BUFS = int(_os.environ.get("KTD2_BUFS", "3"))
KEYT_BUFS = int(_os.environ.get("KEYT_BUFS", "2"))
QFOLD = int(_os.environ.get("QFOLD", "0"))
PREP_DMA = int(_os.environ.get("PREP_DMA", "0"))
NPE_G = int(_os.environ.get("NPE_G", "5"))    # b3 granules on the PE route
QROUND = int(_os.environ.get("QROUND", "0"))  # f32r query-reduce


def build_bass():
    nc = bacc.Bacc(None, target_bir_lowering=False, debug=False)

    query = nc.dram_tensor("query", [BPC, LQ, H], f32, kind="ExternalInput").ap()
    key = nc.dram_tensor("key", [BPC, LK, H], f32, kind="ExternalInput").ap()
    Wq = nc.dram_tensor("Wq", [H, A], f32, kind="ExternalInput").ap()
    bq = nc.dram_tensor("bq", [A], f32, kind="ExternalInput").ap()
    Wk = nc.dram_tensor("Wk", [H, A], f32, kind="ExternalInput").ap()
    bk = nc.dram_tensor("bk", [A], f32, kind="ExternalInput").ap()
    Wv = nc.dram_tensor("Wv", [A, 1], f32, kind="ExternalInput").ap()
    bv = nc.dram_tensor("bv", [1], f32, kind="ExternalInput").ap()
    out = nc.dram_tensor("out", [BPC, 1], f32, kind="ExternalOutput").ap()
    scr_qs = nc.dram_tensor("scr_qs", [H], f32, kind="Internal").ap()
    scr_qp = nc.dram_tensor("scr_qp", [A], f32, kind="Internal").ap()
    scr_u = nc.dram_tensor("scr_u", [H], f32, kind="Internal").ap()

    with tile.TileContext(nc) as tc:
        _build_body(nc, tc, query, key, Wq, bq, Wk, bk, Wv, bv, out,
                    scr_qs, scr_qp, scr_u)
    nc.compile()
    return nc


def _build_body(nc, tc, query, key, Wq, bq, Wk, bk, Wv, bv, out,
                scr_qs, scr_qp, scr_u):
    from contextlib import ExitStack
    ctx = ExitStack()
    with ctx:
        sbc = ctx.enter_context(tc.tile_pool(name="sbc", bufs=1))
        sbq = ctx.enter_context(tc.tile_pool(name="sbq", bufs=1))
        sbk = ctx.enter_context(tc.tile_pool(name="sbk", bufs=1))
        sbr = ctx.enter_context(tc.tile_pool(name="sbr", bufs=1))
        sbsv = ctx.enter_context(tc.tile_pool(name="sbsv", bufs=1))
        sbj = ctx.enter_context(tc.tile_pool(name="sbj", bufs=1))
        sbsm = ctx.enter_context(tc.tile_pool(name="sbsm", bufs=1))
        ps_qr = ctx.enter_context(tc.tile_pool(name="ps_qr", bufs=1, space="PSUM"))
        ps_kt = ctx.enter_context(tc.tile_pool(name="ps_kt", bufs=2, space="PSUM"))
        ps_s2 = ctx.enter_context(tc.tile_pool(name="ps_s2", bufs=2, space="PSUM"))
        ps_sm = ctx.enter_context(tc.tile_pool(name="ps_sm", bufs=1, space="PSUM"))

        # ---------------- constants ----------------
        ident = sbc.tile([P, P], f32)
        colidx = sbsm.tile([P, P], f32, tag="small")
        rowidx = sbsm.tile([P, 1], f32, tag="tiny")
        nc.gpsimd.iota(colidx[:], pattern=[[1, P]], base=0, channel_multiplier=0,
                       allow_small_or_imprecise_dtypes=True)
        nc.gpsimd.iota(rowidx[:], pattern=[[0, 1]], base=0, channel_multiplier=1,
                       allow_small_or_imprecise_dtypes=True)
        nc.vector.tensor_scalar(out=ident[:], in0=colidx[:], scalar1=rowidx[:],
                                scalar2=None, op0=mybir.AluOpType.is_equal)
        ones1 = sbc.tile([P, 1], f32)
        nc.vector.memset(ones1[:], 1.0)
        one11 = sbc.tile([1, 1], f32)
        nc.vector.memset(one11[:], 1.0)
        ones_k1 = sbc.tile([1, P], f32)
        nc.vector.memset(ones_k1[:], 1.0)
        ones_r = sbc.tile([P, 1], f32r)
        nc.vector.tensor_copy(ones_r[:], ones1[:])

        # -------- weight DMAs (ACT queue, first) --------
        # raw Wk/Wq stage through the ktd01 ring (slots recycled for keys)
        wk_st = [sbk.tile([P, 4 * A], f32, tag="ktd01", bufs=KTD01_BUFS,
                          name=f"wkst{i}") for i in range(2)]
        for i in range(2):
            nc.scalar.dma_start(
                out=wk_st[i][:].rearrange("p (j a) -> p j a", j=4),
                in_=Wk[i * A:(i + 1) * A, :].rearrange("(j p) a -> p j a", p=P))
        wq_st = [sbk.tile([P, 4 * A], f32, tag="ktd01", bufs=KTD01_BUFS,
                          name=f"wqst{i}") for i in range(2)]
        for i in range(2):
            nc.scalar.dma_start(
                out=wq_st[i][:].rearrange("p (j a) -> p j a", j=4),
                in_=Wq[i * A:(i + 1) * A, :].rearrange("(j p) a -> p j a", p=P))
        wv_sb = sbc.tile([P, AC], f32)
        nc.scalar.dma_start(out=wv_sb[:].rearrange("p (c o) -> p c o", c=AC),
                            in_=Wv.rearrange("(c p) o -> p c o", p=P))
        bk_sb = sbc.tile([P, AC], f32)
        nc.scalar.dma_start(out=bk_sb[:], in_=bk.rearrange("(c p) -> p c", p=P))
        bv_sb = sbc.tile([1, 1], f32)
        nc.scalar.dma_start(out=bv_sb[:], in_=bv[None, :])
        bq_row = sbc.tile([1, A], f32)
        nc.scalar.dma_start(out=bq_row[:], in_=bq[None, :])

        # -------- one-time weight prep (PE+ACT+DVE, before query issues) ----
        # WkT_r [a-part, (c h)] f32r, rounded at the PSUM->SBUF copies
        WkT_r = sbc.tile([P, AC * H], f32r)
        for c in range(AC):
            for half in range(2):
                wkp = ps_kt.tile([P, A], f32, tag="ktp")
                for jl in range(4):
                    nc.tensor.transpose(
                        wkp[:, jl * P:(jl + 1) * P],
                        wk_st[half][:, jl * A + c * P:jl * A + (c + 1) * P],
                        ident[:])
                nc.scalar.copy(
                    WkT_r[:, c * H + half * A:c * H + (half + 1) * A], wkp[:])
        # Wq_r [h-part, (j a)] f32r
        Wq_r = sbc.tile([P, HJ * A], f32r)
        for i in range(2):
            nc.scalar.copy(Wq_r[:, i * 4 * A:(i + 1) * 4 * A], wq_st[i][:])
        # wv_r, then w row = Wv^T @ WkT  [1, H] fp32
        wv_r = sbc.tile([P, AC], f32r)
        nc.scalar.copy(wv_r[:], wv_sb[:])
        w_sb = sbr.tile([1, H], f32, tag="u", bufs=1, name="w_sb")
        for half in range(2):
            w_ps = ps_sm.tile([1, A], f32, tag="small")
            for c in range(AC):
                nc.tensor.matmul(w_ps[:], wv_r[:, c:c + 1],
                                 WkT_r[:, c * H + half * A:c * H + (half + 1) * A],
                                 start=(c == 0), stop=(c == AC - 1))
            nc.scalar.copy(w_sb[:, half * A:(half + 1) * A], w_ps[:])
        # wvbc [P, H] = broadcast w across partitions (DVE route operand)
        wvbc = sbc.tile([P, H], f32)
        for half in range(2):
            bc_ps = ps_sm.tile([P, A], f32, tag="small")
            nc.tensor.matmul(bc_ps[:], ones_k1[:], w_sb[:, half * A:(half + 1) * A],
                             start=True, stop=True)
            nc.scalar.copy(wvbc[:, half * A:(half + 1) * A], bc_ps[:])
        # wT8 [128, 8]: w chunks as columns (for the b3 w2 tile)
        wT8_ps = ps_sm.tile([P, HJ], f32, tag="small")
        for j in range(HJ):
            nc.tensor.matmul(wT8_ps[:, j:j + 1], w_sb[:, j * P:(j + 1) * P],
                             one11[:], start=True, stop=True)
        wT8 = sbc.tile([P, HJ], f32)
        nc.scalar.copy(wT8[:], wT8_ps[:])
        # cv = bk . Wv + bv
        junk4 = sbsm.tile([P, AC], f32, tag="tiny2")
        cvcol = sbsm.tile([P, 1], f32, tag="tiny3")
        nc.vector.scalar_tensor_tensor(out=junk4[:], in0=bk_sb[:], scalar=1.0,
                                       in1=wv_sb[:], op0=mybir.AluOpType.mult,
                                       op1=mybir.AluOpType.mult, accum_out=cvcol[:])
        cv_ps = ps_sm.tile([1, 1], f32, tag="small")
        nc.tensor.matmul(cv_ps[:], cvcol[:], ones1[:], start=True, stop=True)
        cv_sb = sbc.tile([1, 1], f32)
        nc.vector.tensor_tensor(out=cv_sb[:], in0=cv_ps[:], in1=bv_sb[:],
                                op=mybir.AluOpType.add)

        # -------- key DMAs for b3/b0/b1 (SYNC queue, granule waves) --------
        kt = {}
        for g in range(NG):
            for b in (3, 0, 1):
                tag, bufs = ("ktpe", KTPE_BUFS) if b == 3 else ("ktd01", KTD01_BUFS)
                t = sbk.tile([P, 2 * H], f32, tag=tag, bufs=bufs, name=f"k{b}_{g}")
                nc.sync.dma_start(
                    out=t[:].rearrange("p (c h) -> p c h", c=2),
                    in_=key[b, g * GR:(g + 1) * GR, :]
                    .rearrange("(c p) h -> p c h", p=P))
                kt[(b, g)] = t

        # ---------------- per-batch state ----------------
        preps = {b: {} for b in range(BPC)}
        sv3 = sbsv.tile([2, NPE_G * GR], f32, tag="sv3")
        sdve = {b: sbsv.tile([P, 2 * NG], f32, tag=f"sd{b}", name=f"sdve{b}") for b in range(3)}
        vdve = {b: sbsv.tile([P, 2 * NG], f32, tag=f"vd{b}", name=f"vdve{b}") for b in range(3)}
        sdve[3] = sbsv.tile([P, 2 * (NG - NPE_G)], f32, tag="sd3", name="sdve3")
        vdve[3] = sbsv.tile([P, 2 * (NG - NPE_G)], f32, tag="vd3", name="vdve3")
        qland = {}

        def emit_qdma(b, lo, hi):
            if lo == 0:
                preps[b]["qr_ps"] = [ps_qr.tile([1, A], f32, tag=f"qr{h}", bufs=1,
                                                 name=f"qrps{b}_{h}")
                                     for h in range(2)]
            for i in range(lo, hi):
                t = sbq.tile([P, 2 * H], f32, tag="qland", bufs=QLAND_BUFS,
                             name=f"q{b}_{i}")
                nc.scalar.dma_start(
                    out=t[:].rearrange("p (c h) -> p c h", c=2),
                    in_=query[b, i * 2 * P:(i + 1) * 2 * P, :]
                    .rearrange("(c p) h -> p c h", p=P))
                qland[(b, i)] = t
            if QROUND:
                for i in range(lo, hi):
                    tt = qland[(b, i)]
                    nc.scalar.copy(tt[:].bitcast(f32r), tt[:])

        def emit_kdma2(glo, ghi):
            """b2 key granules on the ACT queue (split around prep2)."""
            for g in range(glo, ghi):
                t = sbk.tile([P, 2 * H], f32, tag="ktd2", bufs=KTD2_BUFS,
                             name=f"k2_{g}")
                nc.scalar.dma_start(
                    out=t[:].rearrange("p (c h) -> p c h", c=2),
                    in_=key[2, g * GR:(g + 1) * GR, :]
                    .rearrange("(c p) h -> p c h", p=P))
                kt[(2, g)] = t

        def emit_qfold(b, lo, hi):
            """GpSimd pair-folds: tile 2i+1 += tile 2i (in place)."""
            for i in range(lo, hi, 2):
                t0, t1 = qland[(b, i)], qland[(b, i + 1)]
                nc.gpsimd.tensor_tensor(out=t1[:], in0=t0[:], in1=t1[:],
                                        op=mybir.AluOpType.add)

        def emit_qr(b, lo, hi):
            """Query-reduce: two [1,512] PSUM groups (h-halves) per batch,
            each accumulating both column-subtiles of the (folded) tiles."""
            step = 2 if QFOLD else 1
            first = 1 if QFOLD else 0
            for i in range(lo + first, hi, step):
                t = qland[(b, i)]
                for half in range(2):
                    for c in range(2):
                        nc.tensor.matmul(
                            preps[b]["qr_ps"][half][:],
                            ones_r[:] if QROUND else ones1[:],
                            t[:, c * H + half * A:c * H + (half + 1) * A]
                            .bitcast(f32r) if QROUND else
                            t[:, c * H + half * A:c * H + (half + 1) * A],
                            start=(i == first and c == 0),
                            stop=(i == NQD - 1 and c == 1))

        def emit_prep(b):
            """qs -> qsT -> qp -> qpT -> u -> (w2 | wqbc) for batch b."""
            d = preps[b]
            qs_sb = sbr.tile([1, H], f32, tag="qs", bufs=1)
            for half in range(2):
                nc.scalar.copy(qs_sb[:, half * A:(half + 1) * A],
                               d["qr_ps"][half][:])
            qsT = sbr.tile([P, HJ], f32r, tag="qsT", bufs=1)
            if PREP_DMA:
                nc.scalar.dma_start(out=scr_qs[None, :], in_=qs_sb[:])
                qsT_st = sbr.tile([P, HJ], f32, tag="qsTst", bufs=1)
                nc.scalar.dma_start(
                    out=qsT_st[:],
                    in_=scr_qs.rearrange("(j p) -> p j", p=P))
                nc.scalar.copy(qsT[:], qsT_st[:])
            else:
                qsT_ps = ps_sm.tile([P, HJ], f32, tag="small")
                for j in range(HJ):
                    nc.tensor.matmul(qsT_ps[:, j:j + 1],
                                     qs_sb[:, j * P:(j + 1) * P], one11[:],
                                     start=True, stop=True)
                nc.scalar.copy(qsT[:], qsT_ps[:])
            qp_ps = ps_sm.tile([1, A], f32, tag="small")
            for j in range(HJ):
                nc.tensor.matmul(qp_ps[:], qsT[:, j:j + 1],
                                 Wq_r[:, j * A:(j + 1) * A],
                                 start=(j == 0), stop=(j == HJ - 1))
            qp_sb = sbr.tile([1, A], f32, tag="qp", bufs=1)
            nc.vector.scalar_tensor_tensor(
                out=qp_sb[:], in0=bq_row[:], scalar=float(LQ), in1=qp_ps[:],
                op0=mybir.AluOpType.mult, op1=mybir.AluOpType.add)
            qpT = sbr.tile([P, AC], f32r, tag="qpT", bufs=1)
            if PREP_DMA:
                nc.scalar.dma_start(out=scr_qp[None, :], in_=qp_sb[:])
                qpT_st = sbr.tile([P, AC], f32, tag="qpTst", bufs=1)
                nc.scalar.dma_start(
                    out=qpT_st[:],
                    in_=scr_qp.rearrange("(c p) -> p c", p=P))
                nc.scalar.copy(qpT[:], qpT_st[:])
            else:
                qpT_ps = ps_sm.tile([P, AC], f32, tag="small")
                for c in range(AC):
                    nc.tensor.matmul(qpT_ps[:, c:c + 1],
                                     qp_sb[:, c * P:(c + 1) * P], one11[:],
                                     start=True, stop=True)
                nc.scalar.copy(qpT[:], qpT_ps[:])
            u_sb = sbr.tile([1, H], f32, tag="u", bufs=1)
            for half in range(2):
                u_ps = ps_sm.tile([1, A], f32, tag="small")
                for c in range(AC):
                    nc.tensor.matmul(
                        u_ps[:], qpT[:, c:c + 1],
                        WkT_r[:, c * H + half * A:c * H + (half + 1) * A],
                        start=(c == 0), stop=(c == AC - 1))
                nc.scalar.copy(u_sb[:, half * A:(half + 1) * A], u_ps[:])
            if b == 3:
                w2 = sbr.tile([P, 2 * HJ], f32r, tag="w2")
                if PREP_DMA:
                    nc.scalar.dma_start(out=scr_u[None, :], in_=u_sb[:])
                    w2_st = sbr.tile([P, 2 * HJ], f32, tag="w2st")
                    nc.scalar.dma_start(
                        out=w2_st[:].rearrange("p (j two) -> p two j", two=2)
                        [:, 0:1, :],
                        in_=scr_u.rearrange("(j p) -> p j", p=P))
                    nc.scalar.copy(
                        w2_st[:].rearrange("p (j two) -> p j two", two=2)
                        [:, :, 1:2],
                        wT8[:].unsqueeze(2))
                    nc.scalar.copy(w2[:], w2_st[:])
                else:
                    w2_ps = ps_sm.tile([P, 2 * HJ], f32, tag="small")
                    for j in range(HJ):
                        nc.tensor.matmul(w2_ps[:, 2 * j:2 * j + 1],
                                         u_sb[:, j * P:(j + 1) * P], one11[:],
                                         start=True, stop=True)
                    nc.scalar.copy(w2[:], w2_ps[:])
                    nc.scalar.copy(
                        w2[:].rearrange("p (j two) -> p j two", two=2)[:, :, 1:2],
                        wT8[:].unsqueeze(2))
                d["w2"] = w2
                wqbc = sbr.tile([P, H], f32, tag="wqbc3")
                for half in range(2):
                    bc_ps = ps_sm.tile([P, A], f32, tag="small")
                    nc.tensor.matmul(bc_ps[:], ones_k1[:],
                                     u_sb[:, half * A:(half + 1) * A],
                                     start=True, stop=True)
                    nc.scalar.copy(wqbc[:, half * A:(half + 1) * A], bc_ps[:])
                d["wqbc"] = wqbc
            else:
                wqbc = sbr.tile([P, H], f32, tag=f"wqbc{b}")
                for half in range(2):
                    bc_ps = ps_sm.tile([P, A], f32, tag="small")
                    nc.tensor.matmul(bc_ps[:], ones_k1[:],
                                     u_sb[:, half * A:(half + 1) * A],
                                     start=True, stop=True)
                    nc.scalar.copy(wqbc[:, half * A:(half + 1) * A], bc_ps[:])
                d["wqbc"] = wqbc

        def emit_route_pe(g):
            """b3 granule g: transposes -> keyT (f32r) -> s2 matmuls -> sv."""
            t = kt[(3, g)]
            keyT = sbk.tile([P, 2 * H], f32r, tag="keyT", bufs=KEYT_BUFS)
            for j in range(HJ):
                ktp = ps_kt.tile([P, 2 * P], f32, tag="ktp")
                for c in range(2):
                    nc.tensor.transpose(ktp[:, c * P:(c + 1) * P],
                                        t[:, c * H + j * P:c * H + (j + 1) * P],
                                        ident[:])
                nc.scalar.copy(keyT[:, j * 2 * P:(j + 1) * 2 * P], ktp[:])
            s2 = ps_s2.tile([2, 2 * P], f32, tag="s2")
            w2 = preps[3]["w2"]
            for j in range(HJ):
                nc.tensor.matmul(s2[:], w2[:, 2 * j:2 * j + 2],
                                 keyT[:, j * 2 * P:(j + 1) * 2 * P],
                                 start=(j == 0), stop=(j == HJ - 1))
            nc.scalar.copy(sv3[:, g * GR:(g + 1) * GR], s2[:])

        def emit_dots_dve(b, g):
            t = kt[(b, g)]
            for c in range(2):
                ti = (g - (NPE_G if b == 3 else 0)) * 2 + c
                j1 = sbj.tile([P, H], f32, tag="junk", bufs=2)
                nc.vector.scalar_tensor_tensor(
                    out=j1[:], in0=t[:, c * H:(c + 1) * H], scalar=1.0,
                    in1=preps[b]["wqbc"][:], op0=mybir.AluOpType.mult,
                    op1=mybir.AluOpType.mult, accum_out=sdve[b][:, ti:ti + 1])
                j2 = sbj.tile([P, H], f32, tag="junk", bufs=2)
                nc.vector.scalar_tensor_tensor(
                    out=j2[:], in0=t[:, c * H:(c + 1) * H], scalar=1.0,
                    in1=wvbc[:], op0=mybir.AluOpType.mult,
                    op1=mybir.AluOpType.mult, accum_out=vdve[b][:, ti:ti + 1])

        def emit_softmax_pe():
            # b3: combine the PE-route part (sv3 rows, granules < NPE_G) with
            # the DVE part (sdve3/vdve3 columns, granules >= NPE_G)
            vsw = sbj.tile([2, NPE_G * GR], f32, tag="vsw")
            nc.vector.stream_shuffle(vsw[:], sv3[:], [1, 0] + list(range(2, 32)))
            smax = sbsm.tile([2, 1], f32, tag="smax")
            nc.vector.reduce_max(smax[:], sv3[:], axis=mybir.AxisListType.X)
            # dve-part max
            m1 = sbsm.tile([P, 1], f32, tag="m1")
            nc.vector.reduce_max(m1[:], sdve[3][:], axis=mybir.AxisListType.X)
            mT_ps = ps_sm.tile([1, P], f32, tag="small")
            nc.tensor.transpose(mT_ps[:], m1[:], ident[:])
            mT_sb = sbsm.tile([1, P], f32, tag="mT")
            nc.vector.tensor_copy(mT_sb[:], mT_ps[:])
            gmax = sbsm.tile([1, 1], f32, tag="gmax")
            nc.vector.reduce_max(gmax[:], mT_sb[:], axis=mybir.AxisListType.X)
            nc.vector.tensor_tensor(out=gmax[:], in0=gmax[:], in1=smax[0:1, :],
                                    op=mybir.AluOpType.max)
            ngm1 = sbsm.tile([1, 1], f32, tag="ngm1")
            nc.vector.tensor_scalar_mul(ngm1[:], gmax[:], -1.0)
            # PE part: exp + num/den
            den = sbsm.tile([1, 1], f32, tag="den")
            nc.scalar.activation(sv3[0:1, :], sv3[0:1, :],
                                 mybir.ActivationFunctionType.Exp,
                                 bias=ngm1[:], scale=1.0, accum_out=den[:])
            num = sbsm.tile([1, 1], f32, tag="num")
            nc.vector.scalar_tensor_tensor(
                out=vsw[0:1, :], in0=sv3[0:1, :], scalar=1.0, in1=vsw[0:1, :],
                op0=mybir.AluOpType.mult, op1=mybir.AluOpType.mult,
                accum_out=num[:])
            # DVE part: exp + num/den (bias = -gmax broadcast)
            ng_ps = ps_sm.tile([P, 1], f32, tag="small")
            nc.tensor.matmul(ng_ps[:], ones_k1[:], gmax[:], start=True, stop=True)
            ngm = sbsm.tile([P, 1], f32, tag="ngm")
            nc.vector.tensor_scalar_mul(ngm[:], ng_ps[:], -1.0)
            e128 = sbsm.tile([P, 2 * (NG - NPE_G)], f32, tag="e1283")
            erow = sbsm.tile([P, 1], f32, tag="erowp")
            nc.scalar.activation(e128[:], sdve[3][:],
                                 mybir.ActivationFunctionType.Exp,
                                 bias=ngm[:], scale=1.0, accum_out=erow[:])
            junk5 = sbsm.tile([P, 2 * (NG - NPE_G)], f32, tag="junk53")
            nrow = sbsm.tile([P, 1], f32, tag="nrow")
            nc.vector.scalar_tensor_tensor(
                out=junk5[:], in0=e128[:], scalar=1.0, in1=vdve[3][:],
                op0=mybir.AluOpType.mult, op1=mybir.AluOpType.mult,
                accum_out=nrow[:])
            den_ps = ps_sm.tile([1, 2], f32, tag="small")
            nc.tensor.matmul(den_ps[:, 0:1], erow[:], ones1[:], start=True, stop=True)
            nc.tensor.matmul(den_ps[:, 1:2], nrow[:], ones1[:], start=True, stop=True)
            dn = sbsm.tile([1, 2], f32, tag="dn")
            nc.vector.tensor_copy(dn[:], den_ps[:])
            tden = sbsm.tile([1, 1], f32, tag="tden")
            nc.vector.tensor_tensor(out=tden[:], in0=den[:], in1=dn[:, 0:1],
                                    op=mybir.AluOpType.add)
            tnum = sbsm.tile([1, 1], f32, tag="tnum")
            nc.vector.tensor_tensor(out=tnum[:], in0=num[:], in1=dn[:, 1:2],
                                    op=mybir.AluOpType.add)
            _emit_final(nc, sbsm, tnum, tden, cv_sb, out, 3)

        def emit_softmax_dve(b):
            m1 = sbsm.tile([P, 1], f32, tag="m1")
            nc.vector.reduce_max(m1[:], sdve[b][:], axis=mybir.AxisListType.X)
            mT_ps = ps_sm.tile([1, P], f32, tag="small")
            nc.tensor.transpose(mT_ps[:], m1[:], ident[:])
            mT_sb = sbsm.tile([1, P], f32, tag="mT")
            nc.vector.tensor_copy(mT_sb[:], mT_ps[:])
            gmax = sbsm.tile([1, 1], f32, tag="gmax")
            nc.vector.reduce_max(gmax[:], mT_sb[:], axis=mybir.AxisListType.X)
            ng_ps = ps_sm.tile([P, 1], f32, tag="small")
            nc.tensor.matmul(ng_ps[:], ones_k1[:], gmax[:], start=True, stop=True)
            ngm = sbsm.tile([P, 1], f32, tag="ngm")
            nc.vector.tensor_scalar_mul(ngm[:], ng_ps[:], -1.0)
            e128 = sbsm.tile([P, 2 * NG], f32, tag="e128")
            erow = sbsm.tile([P, 1], f32, tag="erowp")
            nc.scalar.activation(e128[:], sdve[b][:],
                                 mybir.ActivationFunctionType.Exp,
                                 bias=ngm[:], scale=1.0, accum_out=erow[:])
            junk5 = sbsm.tile([P, 2 * NG], f32, tag="junk5")
            nrow = sbsm.tile([P, 1], f32, tag="nrow")
            nc.vector.scalar_tensor_tensor(
                out=junk5[:], in0=e128[:], scalar=1.0, in1=vdve[b][:],
                op0=mybir.AluOpType.mult, op1=mybir.AluOpType.mult,
                accum_out=nrow[:])
            den_ps = ps_sm.tile([1, 2], f32, tag="small")
            nc.tensor.matmul(den_ps[:, 0:1], erow[:], ones1[:], start=True, stop=True)
            nc.tensor.matmul(den_ps[:, 1:2], nrow[:], ones1[:], start=True, stop=True)
            dn = sbsm.tile([1, 2], f32, tag="dn")
            nc.vector.tensor_copy(dn[:], den_ps[:])
            _emit_final(nc, sbsm, dn[:, 1:2], dn[:, 0:1], cv_sb, out, b)

        # ---------------- main emission interleave ----------------
        # Call order defines each engine's in-order stream; comments give the
        # intended wall-clock position.
        emit_qdma(3, 0, NQD)                 # q3 streams [0, ~28us]
        if QFOLD:
            emit_qfold(3, 0, NQD)
        emit_qr(3, 0, NQD)
        emit_prep(3)                         # ~30us
        emit_route_pe(0)
        emit_qdma(0, 0, 4)
        emit_route_pe(1)
        emit_qdma(0, 4, NQD)
        if QFOLD:
            emit_qfold(0, 0, NQD)
        emit_qr(0, 0, NQD)
        emit_prep(0)                         # ~58us
        emit_dots_dve(0, 0)
        emit_route_pe(2)
        emit_qdma(1, 0, 4)
        emit_dots_dve(0, 1)
        emit_route_pe(3)
        emit_qdma(1, 4, NQD)
        if QFOLD:
            emit_qfold(1, 0, NQD)
        emit_qr(1, 0, NQD)
        emit_prep(1)                         # ~85us
        emit_dots_dve(1, 0)
        emit_dots_dve(1, 1)
        emit_route_pe(4)
        emit_qdma(2, 0, 4)
        emit_kdma2(0, 3)                     # b2 early granules (ring-bounded)
        emit_dots_dve(0, 2)
        emit_dots_dve(1, 2)
        emit_dots_dve(3, 5)
        emit_qdma(2, 4, NQD)
        if QFOLD:
            emit_qfold(2, 0, NQD)
        emit_qr(2, 0, NQD)
        emit_prep(2)                         # ~110us
        emit_kdma2(3, NG)                    # rest of b2 (after prep2 copies)
        emit_dots_dve(2, 0)
        emit_dots_dve(2, 1)
        emit_dots_dve(2, 2)
        emit_dots_dve(3, 6)
        for g in range(3, NG):
            emit_dots_dve(0, g)
            emit_dots_dve(1, g)
            emit_dots_dve(2, g)
            if g == 3:
                emit_dots_dve(3, 7)
        emit_softmax_pe()
        for b in range(3):
            emit_softmax_dve(b)


def _emit_final(nc, sbsm, num, den, cv_sb, out, b):
    rden = sbsm.tile([1, 1], f32, tag="rden")
    nc.vector.reciprocal(rden[:], den[:])
    x = sbsm.tile([1, 1], f32, tag="x")
    nc.vector.tensor_tensor(out=x[:], in0=num[:], in1=rden[:],
                            op=mybir.AluOpType.mult)
    x2 = sbsm.tile([1, 1], f32, tag="x2")
    nc.vector.tensor_tensor(out=x2[:], in0=x[:], in1=cv_sb[:],
                            op=mybir.AluOpType.add)
    nc.sync.dma_start(out=out[b:b + 1, :], in_=x2[:])


def _shard(query, key, shared):
    in_maps = []
    for c in range(N_CORES):
        sl = slice(c * BPC, (c + 1) * BPC)
        m = {"query": np.ascontiguousarray(query[sl]),
             "key": np.ascontiguousarray(key[sl])}
        m.update(shared)
        in_maps.append(m)
    return in_maps


def _make_in_maps(inputs):
    query = np.ascontiguousarray(np.asarray(inputs["query"], dtype=np.float32))
    key = np.ascontiguousarray(np.asarray(inputs["key"], dtype=np.float32))
    shared = {k: np.ascontiguousarray(np.asarray(inputs[k], dtype=np.float32))
              for k in ("Wq", "bq", "Wk", "bk", "Wv", "bv")}
    return _shard(query, key, shared)


def kernel(**inputs):
    if "nc" not in _CACHE:
        _CACHE["nc"] = build_bass()
    nc = _CACHE["nc"]
    in_maps = _make_in_maps(inputs)
    res = run_bass_kernel_spmd(nc, in_maps, list(range(N_CORES)))
    outs = [res.results[c]["out"] for c in range(N_CORES)]
    return np.concatenate(outs, axis=0).astype(np.float32)


if __name__ == "__main__":
    rng = np.random.default_rng(0)
    ins = {
        "query": rng.standard_normal((B, LQ, H), dtype=np.float32),
        "key": rng.standard_normal((B, LK, H), dtype=np.float32),
        "Wq": (rng.standard_normal((H, A), dtype=np.float32) / np.sqrt(H)).astype(np.float32),
        "bq": np.zeros((A,), np.float32),
        "Wk": (rng.standard_normal((H, A), dtype=np.float32) / np.sqrt(H)).astype(np.float32),
        "bk": np.zeros((A,), np.float32),
        "Wv": (rng.standard_normal((A, 1), dtype=np.float32) / np.sqrt(A)).astype(np.float32),
        "bv": np.zeros((1,), np.float32),
    }
    x = kernel(**ins)
    print("kernel out:", x[:4, 0])
